# revision 18
# baseline (speedup 1.0000x reference)
"""Trainium2 Bass kernel for nn_Attention_layer_67877663146058.

Computes attn = softmax((x @ W_qkv.T)[q] @ (x @ W_qkv.T)[k]^T * hd**-0.5)
for x [8, 1024, 768], W_qkv [2304, 768] -> out [8, 12, 1024, 1024] fp32.

Sharding: batch-parallel across the 8 NeuronCores (core b handles batch b,
all 12 heads). The V third of the QKV projection never reaches the output,
so only the Q and K rows of W_qkv are used.

Layout strategy: the PE contracts over the partition dim of both operands,
so the projection needs x^T [e, n] and W^T [e, f] — both produced on the
host (cheap numpy transposes during input prep; DMA transpose on TRN2 is
2-byte-dtype-only). The projection output Q^T/K^T [f, n] is then exactly
the [d, n] layout the scores matmul wants for both operands.

Matmuls run as float32r (same fp32 bytes, faster PE mode: 1 cycle/row vs
2-4 for plain fp32). The two heads that share an f-tile occupy PE row
groups 0:64 / 64:128 via tile_position so their K=64 score matmuls overlap.

Softmax skips the max-subtraction (scores are ~N(0,1) after the 1/8 scale;
exp never overflows fp32) so the only per-element passes are:
  PE matmul -> PSUM, ACT exp (+free row-sum accumulator) -> SBUF,
  DVE per-row scale -> SBUF, DMA -> HBM.
"""

import numpy as np
from contextlib import ExitStack

import concourse.bacc as bacc
import concourse.mybir as mybir
import concourse.tile as tile

# bass_utils imports antenv.axon_hooks when BASS_TRACE is set in the
# environment; some images ship an antenv stub without that module. Register
# a no-op fallback so tracing degrades gracefully instead of crashing.
try:
    from antenv.axon_hooks import get_axon_ntff_profile_hook as _g  # noqa: F401
except Exception:
    import sys as _sys
    import types as _types

    _m = _types.ModuleType("antenv.axon_hooks")
    _state = {"h": None}
    _m.set_axon_ntff_profile_hook = lambda h: _state.__setitem__("h", h)
    _m.get_axon_ntff_profile_hook = lambda: _state["h"]
    _sys.modules["antenv.axon_hooks"] = _m
    try:
        import antenv as _antenv

        _antenv.axon_hooks = _m
    except Exception:
        pass

from concourse.bass_utils import run_bass_kernel_spmd

B = 8          # batches == cores
N = 1024       # tokens
E = 768        # embed dim
H = 12         # heads
HD = 64        # head dim
F = H * HD     # 768 features per projection (Q or K)
ET = E // 128  # 6 e-tiles
FT = F // 128  # 6 f-tiles (2 heads per f-tile)
QB = N // 128  # 8 query blocks
SCALE = HD ** -0.5

_cache = {}


def _build(use_f32r=True):
    f32 = mybir.dt.float32
    mm_dt = mybir.dt.float32r if use_f32r else f32
    nc = bacc.Bacc("TRN2", debug=False, num_devices=B)

    xT_d = nc.dram_tensor("xT", [E, N], f32, kind="ExternalInput")
    wT_d = nc.dram_tensor("wT", [E, 2 * F], f32, kind="ExternalInput")
    out_d = nc.dram_tensor("out", [H, N, N], f32, kind="ExternalOutput")

    xT_src = xT_d.ap().rearrange("(t p) n -> t p n", p=128)       # [6,128,1024]
    wT_src = wT_d.ap().rearrange("(t p) f -> t p f", p=128)       # [6,128,1536]
    out_flat = out_d.ap().rearrange("h q n -> (h q) n")           # [12288,1024]

    def mm(out_ap, lhsT, rhs, **kw):
        nc.tensor.matmul(out_ap, lhsT, rhs, **kw)

    with ExitStack() as ctx:
        tc = ctx.enter_context(tile.TileContext(nc))
        statics = ctx.enter_context(tc.tile_pool(name="statics", bufs=1))
        work = ctx.enter_context(tc.tile_pool(name="work", bufs=8))
        small = ctx.enter_context(tc.tile_pool(name="small", bufs=8))
        pproj = ctx.enter_context(tc.tile_pool(name="pproj", bufs=2, space="PSUM"))
        pscore = ctx.enter_context(tc.tile_pool(name="pscore", bufs=2, space="PSUM"))

        xt = statics.tile([128, ET, N], mm_dt, tag="xt", name="xt")
        wt = statics.tile([128, ET, 2 * F], mm_dt, tag="wt", name="wt")
        qt = statics.tile([128, FT, N], mm_dt, tag="qt", name="qt")
        kt = statics.tile([128, FT, N], mm_dt, tag="kt", name="kt")

        # Preload the exp table set while input DMAs run: a dependency-free
        # dummy ACTIVATE at t=0 pulls the ~2.7us ACT_TABLE_LOAD off the
        # critical path of the first real exp.
        warm = small.tile([128, 1], f32, tag="sums", name="warm")
        nc.vector.memset(warm, 0.0)
        nc.scalar.activation(warm, warm, mybir.ActivationFunctionType.Exp)

        # Input loads, chunked per e-tile so the first projection matmuls can
        # start as soon as the first chunks land.
        # Single sync-ring FIFO, priority-ordered: x chunks and the W columns
        # for f-tiles 0-1 first (they gate projections 0-1), then the rest in
        # f-tile order. 512-col chunks keep DMA descriptor runs at 2KB.
        for ei in range(ET):
            nc.sync.dma_start(xt[:, ei, :], xT_src[ei].bitcast(mm_dt))
            nc.sync.dma_start(wt[:, ei, 0:256], wT_src[ei][:, 0:256].bitcast(mm_dt))
        for ei in range(ET):
            nc.sync.dma_start(wt[:, ei, 256:512], wT_src[ei][:, 256:512].bitcast(mm_dt))
        for fg in range(1, 3):
            c0, c1 = fg * 512, (fg + 1) * 512
            for ei in range(ET):
                nc.sync.dma_start(
                    wt[:, ei, c0:c1], wT_src[ei][:, c0:c1].bitcast(mm_dt)
                )

        def emit_proj(fi):
            # qT/kT tile fi = W^T-cols.T @ x^T  ([128, 1024] each)
            pq = pproj.tile([128, N], f32, tag="proj", name=f"pq{fi}")
            pk = pproj.tile([128, N], f32, tag="proj", name=f"pk{fi}")
            for dst, foff in ((pq, 2 * fi * 128), (pk, (2 * fi + 1) * 128)):
                for ei in range(ET):
                    for nh in range(2):
                        mm(
                            dst[:, nh * 512:(nh + 1) * 512],
                            lhsT=wt[:, ei, foff:foff + 128],
                            rhs=xt[:, ei, nh * 512:(nh + 1) * 512],
                            start=(ei == 0),
                            stop=(ei == ET - 1),
                        )
            # all copies on DVE (ACT is the steady-state co-bottleneck);
            # kt first (gates every scores rhs), then the qb0 slice of qt so
            # the first scores matmul of the stage unblocks early.
            nc.vector.tensor_copy(kt[:, fi, :], pk)
            nc.vector.tensor_copy(qt[:, fi, 0:128], pq[:, 0:128])
            nc.vector.tensor_copy(qt[:, fi, 128:N], pq[:, 128:N])

        def emit_attn(fi):
            # scores + softmax for the two heads in this f-tile. Head 2fi
            # lives in partitions 0:64, head 2fi+1 in 64:128 -> their K=64
            # matmuls target different PE row groups and run concurrently.
            for qb in range(QB):
                scores = [
                    pscore.tile([128, N], f32, tag="ps", name=f"ps{fi}_{qb}_{hh}")
                    for hh in range(2)
                ]
                for hh in range(2):
                    for nh in range(2):
                        lo, hi = hh * 64, hh * 64 + 64
                        mm(
                            scores[hh][:, nh * 512:(nh + 1) * 512],
                            lhsT=qt[lo:hi, fi, qb * 128:(qb + 1) * 128],
                            rhs=kt[lo:hi, fi, nh * 512:(nh + 1) * 512],
                            start=True,
                            stop=True,
                            tile_position=(hh * 64, 0),
                        )
                for hh in range(2):
                    h = 2 * fi + hh
                    ot = work.tile([128, N], f32, tag="out", name=f"ot{fi}_{qb}_{hh}")
                    sums = small.tile([128, 1], f32, tag="sums", name=f"sm{fi}_{qb}_{hh}")
                    nc.scalar.activation(
                        ot, scores[hh], mybir.ActivationFunctionType.Exp,
                        scale=SCALE, accum_out=sums,
                    )
                    rec = small.tile([128, 1], f32, tag="rec", name=f"rc{fi}_{qb}_{hh}")
                    nc.vector.reciprocal(rec, sums)
                    nc.vector.tensor_scalar_mul(ot, ot, rec)
                    nc.sync.dma_start(
                        out_flat[h * N + qb * 128:h * N + (qb + 1) * 128], ot
                    )

        for fi in range(FT):
            emit_proj(fi)
            emit_attn(fi)

    nc.compile()
    return nc


def _run(x, W_qkv, trace=False, use_f32r=True):
    key = ("nc", use_f32r)
    if key not in _cache:
        _cache[key] = _build(use_f32r)
    nc = _cache[key]

    x = np.asarray(x, dtype=np.float32)
    W_qkv = np.asarray(W_qkv, dtype=np.float32)
    # interleave Q/K 128-col blocks per f-tile: [Q0,K0,Q1,K1,...,Q5,K5]
    wqk = W_qkv[: 2 * F].reshape(2, FT, 128, E)           # [qk, fi, 128, e]
    wqk = wqk.transpose(3, 1, 0, 2).reshape(E, 2 * F)     # [e, fi*qk*128]
    wT = np.ascontiguousarray(wqk)                        # [768, 1536]
    in_maps = [
        {"xT": np.ascontiguousarray(x[b].T), "wT": wT}
        for b in range(B)
    ]
    res = run_bass_kernel_spmd(nc, in_maps, core_ids=list(range(B)), trace=trace)
    out = np.stack([r["out"] for r in res.results], axis=0)
    return out, res


def kernel(x, W_qkv):
    return _run(x, W_qkv)[0]


# revision 19
# speedup vs baseline: 1.0826x; 1.0826x over previous
"""Trainium2 Bass kernel for nn_Attention_layer_67877663146058.

Computes attn = softmax((x @ W_qkv.T)[q] @ (x @ W_qkv.T)[k]^T * hd**-0.5)
for x [8, 1024, 768], W_qkv [2304, 768] -> out [8, 12, 1024, 1024] fp32.

Sharding: batch-parallel across the 8 NeuronCores (core b handles batch b,
all 12 heads). The V third of the QKV projection never reaches the output,
so only the Q and K rows of W_qkv are used.

Layout strategy: the PE contracts over the partition dim of both operands,
so the projection needs x^T [e, n] and W^T [e, f] — both produced on the
host (cheap numpy transposes during input prep; DMA transpose on TRN2 is
2-byte-dtype-only). The projection output Q^T/K^T [f, n] is then exactly
the [d, n] layout the scores matmul wants for both operands.

Matmuls run as float32r (same fp32 bytes, faster PE mode: 1 cycle/row vs
2-4 for plain fp32). The two heads that share an f-tile occupy PE row
groups 0:64 / 64:128 via tile_position so their K=64 score matmuls overlap.

Softmax skips the max-subtraction (scores are ~N(0,1) after the 1/8 scale;
exp never overflows fp32) so the only per-element passes are:
  PE matmul -> PSUM, ACT exp (+free row-sum accumulator) -> SBUF,
  DVE per-row scale -> SBUF, DMA -> HBM.
"""

import numpy as np
from contextlib import ExitStack

import concourse.bacc as bacc
import concourse.mybir as mybir
import concourse.tile as tile

# bass_utils imports antenv.axon_hooks when BASS_TRACE is set in the
# environment; some images ship an antenv stub without that module. Register
# a no-op fallback so tracing degrades gracefully instead of crashing.
try:
    from antenv.axon_hooks import get_axon_ntff_profile_hook as _g  # noqa: F401
except Exception:
    import sys as _sys
    import types as _types

    _m = _types.ModuleType("antenv.axon_hooks")
    _state = {"h": None}
    _m.set_axon_ntff_profile_hook = lambda h: _state.__setitem__("h", h)
    _m.get_axon_ntff_profile_hook = lambda: _state["h"]
    _sys.modules["antenv.axon_hooks"] = _m
    try:
        import antenv as _antenv

        _antenv.axon_hooks = _m
    except Exception:
        pass

from concourse.bass_utils import run_bass_kernel_spmd

B = 8          # batches == cores
N = 1024       # tokens
E = 768        # embed dim
H = 12         # heads
HD = 64        # head dim
F = H * HD     # 768 features per projection (Q or K)
ET = E // 128  # 6 e-tiles
FT = F // 128  # 6 f-tiles (2 heads per f-tile)
QB = N // 128  # 8 query blocks
SCALE = HD ** -0.5

_cache = {}


def _build(use_f32r=True):
    f32 = mybir.dt.float32
    mm_dt = mybir.dt.float32r if use_f32r else f32
    nc = bacc.Bacc("TRN2", debug=False, num_devices=B)

    xT_d = nc.dram_tensor("xT", [E, N], f32, kind="ExternalInput")
    wT_d = nc.dram_tensor("wT", [E, 2 * F], f32, kind="ExternalInput")
    out_d = nc.dram_tensor("out", [H, N, N], f32, kind="ExternalOutput")

    xT_src = xT_d.ap().rearrange("(t p) n -> t p n", p=128)       # [6,128,1024]
    wT_src = wT_d.ap().rearrange("(t p) f -> t p f", p=128)       # [6,128,1536]
    out_flat = out_d.ap().rearrange("h q n -> (h q) n")           # [12288,1024]

    def mm(out_ap, lhsT, rhs, **kw):
        nc.tensor.matmul(out_ap, lhsT, rhs, **kw)

    with ExitStack() as ctx:
        tc = ctx.enter_context(tile.TileContext(nc))
        statics = ctx.enter_context(tc.tile_pool(name="statics", bufs=1))
        work = ctx.enter_context(tc.tile_pool(name="work", bufs=8))
        small = ctx.enter_context(tc.tile_pool(name="small", bufs=8))
        pproj = ctx.enter_context(tc.tile_pool(name="pproj", bufs=2, space="PSUM"))
        pscore = ctx.enter_context(tc.tile_pool(name="pscore", bufs=3, space="PSUM"))

        xt = statics.tile([128, ET, N], mm_dt, tag="xt", name="xt")
        wt = statics.tile([128, ET, 2 * F], mm_dt, tag="wt", name="wt")
        qt = statics.tile([128, FT, N], mm_dt, tag="qt", name="qt")
        kt = statics.tile([128, FT, N], mm_dt, tag="kt", name="kt")

        # Preload the exp table set while input DMAs run: a dependency-free
        # dummy ACTIVATE at t=0 pulls the ~2.7us ACT_TABLE_LOAD off the
        # critical path of the first real exp.
        warm = small.tile([128, 1], f32, tag="sums", name="warm")
        nc.vector.memset(warm, 0.0)
        nc.scalar.activation(warm, warm, mybir.ActivationFunctionType.Exp)

        # Input loads, chunked per e-tile so the first projection matmuls can
        # start as soon as the first chunks land.
        # Single sync-ring FIFO, priority-ordered: x chunks and the W columns
        # for f-tiles 0-1 first (they gate projections 0-1), then the rest in
        # f-tile order. 512-col chunks keep DMA descriptor runs at 2KB.
        for ei in range(ET):
            nc.sync.dma_start(xt[:, ei, :], xT_src[ei].bitcast(mm_dt))
            nc.sync.dma_start(wt[:, ei, 0:256], wT_src[ei][:, 0:256].bitcast(mm_dt))
        for ei in range(ET):
            nc.sync.dma_start(wt[:, ei, 256:512], wT_src[ei][:, 256:512].bitcast(mm_dt))
        for fg in range(1, 3):
            c0, c1 = fg * 512, (fg + 1) * 512
            for ei in range(ET):
                nc.sync.dma_start(
                    wt[:, ei, c0:c1], wT_src[ei][:, c0:c1].bitcast(mm_dt)
                )

        def emit_proj(fi):
            # qT/kT tile fi = W^T-cols.T @ x^T, as four single-bank [128,512]
            # accumulation tiles so projection holds only 2 PSUM banks
            # (bufs=2 keeps copy-read and next-group matmul-write in
            # disjoint banks), freeing banks for deeper scores buffering.
            # K halves first: kt gates every scores rhs.
            for dst, foff, nh in (
                (kt, (2 * fi + 1) * 128, 0),
                (kt, (2 * fi + 1) * 128, 1),
                (qt, 2 * fi * 128, 0),
                (qt, 2 * fi * 128, 1),
            ):
                pt = pproj.tile([128, 512], f32, tag="proj",
                                name=f"pp{fi}_{foff}_{nh}")
                for ei in range(ET):
                    mm(
                        pt,
                        lhsT=wt[:, ei, foff:foff + 128],
                        rhs=xt[:, ei, nh * 512:(nh + 1) * 512],
                        start=(ei == 0),
                        stop=(ei == ET - 1),
                    )
                nc.vector.tensor_copy(dst[:, fi, nh * 512:(nh + 1) * 512], pt)

        def emit_attn(fi):
            # scores + softmax for the two heads in this f-tile. Head 2fi
            # lives in partitions 0:64, head 2fi+1 in 64:128 -> their K=64
            # matmuls target different PE row groups and run concurrently.
            for qb in range(QB):
                scores = [
                    pscore.tile([128, N], f32, tag="ps", name=f"ps{fi}_{qb}_{hh}")
                    for hh in range(2)
                ]
                for hh in range(2):
                    for nh in range(2):
                        lo, hi = hh * 64, hh * 64 + 64
                        mm(
                            scores[hh][:, nh * 512:(nh + 1) * 512],
                            lhsT=qt[lo:hi, fi, qb * 128:(qb + 1) * 128],
                            rhs=kt[lo:hi, fi, nh * 512:(nh + 1) * 512],
                            start=True,
                            stop=True,
                            tile_position=(hh * 64, 0),
                        )
                for hh in range(2):
                    h = 2 * fi + hh
                    ot = work.tile([128, N], f32, tag="out", name=f"ot{fi}_{qb}_{hh}")
                    sums = small.tile([128, 1], f32, tag="sums", name=f"sm{fi}_{qb}_{hh}")
                    nc.scalar.activation(
                        ot, scores[hh], mybir.ActivationFunctionType.Exp,
                        scale=SCALE, accum_out=sums,
                    )
                    rec = small.tile([128, 1], f32, tag="rec", name=f"rc{fi}_{qb}_{hh}")
                    nc.vector.reciprocal(rec, sums)
                    nc.vector.tensor_scalar_mul(ot, ot, rec)
                    nc.sync.dma_start(
                        out_flat[h * N + qb * 128:h * N + (qb + 1) * 128], ot
                    )

        for fi in range(FT):
            emit_proj(fi)
            emit_attn(fi)

    nc.compile()
    return nc


def _run(x, W_qkv, trace=False, use_f32r=True):
    key = ("nc", use_f32r)
    if key not in _cache:
        _cache[key] = _build(use_f32r)
    nc = _cache[key]

    x = np.asarray(x, dtype=np.float32)
    W_qkv = np.asarray(W_qkv, dtype=np.float32)
    # interleave Q/K 128-col blocks per f-tile: [Q0,K0,Q1,K1,...,Q5,K5]
    wqk = W_qkv[: 2 * F].reshape(2, FT, 128, E)           # [qk, fi, 128, e]
    wqk = wqk.transpose(3, 1, 0, 2).reshape(E, 2 * F)     # [e, fi*qk*128]
    wT = np.ascontiguousarray(wqk)                        # [768, 1536]
    in_maps = [
        {"xT": np.ascontiguousarray(x[b].T), "wT": wT}
        for b in range(B)
    ]
    res = run_bass_kernel_spmd(nc, in_maps, core_ids=list(range(B)), trace=trace)
    out = np.stack([r["out"] for r in res.results], axis=0)
    return out, res


def kernel(x, W_qkv):
    return _run(x, W_qkv)[0]
